# revision 11
# baseline (speedup 1.0000x reference)
"""AttentionBlock kernel for 8 Trainium2 NeuronCores.

Reference computation (per batch b):
    Q = x[b] @ Wq + bq            [S, D]
    K = x[b] @ Wk + bk            [S, D]
    V = x[b] @ Wv + bv            [S, D]
    scores = Q @ K^T              [S, S]   (unscaled)
    attn = softmax(scores, -1)
    out[b] = attn @ V / sqrt(D)

Sharding: 8 cores = 4 batches x 2 sequence-halves. Each core projects
Q/K/V only for its OWN 1024 rows, then the K-half (f32r) and V-half (bf16)
are exchanged within the pair via a 2-way AllGather (DRAM bounce), so the
duplicated K/V projection work of the previous version is gone (-22% PE
work). Attention (scores/softmax/attn@V) runs per-core over the core's
1024 queries x full 2048 keys. AllGather output is in group order
(even-core half first), identical on both cores, so the program stays
SPMD-uniform; key order matches the natural sequence order.

Precision: all score-path operands (x, Wq, Wk, Wv, K) stay f32r. Host
pre-rounds x/W to 13 mantissa bits so the device f32->f32r interpretation
is lossless, which lets DMAs land f32 bytes directly into f32r tiles (no
DVE casts anywhere). eT (=exp scores) and V are bf16 (output-path only,
~1e-3 error), halving their SBUF/DMA cost; K stays fully SBUF-resident
[128, 8, 2048] f32r so there is no K DRAM stream during scores.

Engine plan: sync ring = input loads (x, wk, wq, wv) then the AllGather-
gated K/V reloads; scalar(ACT) = evictions + inline output stores;
gpsimd = exchange bounce stores + the two collectives; DVE nearly idle
(reciprocal + bias adds only). rowsum rides a ones(=32)-matmul on the PE
(folds the 1/sqrt(d_k)=1/32 scale), reciprocal on DVE, PE-transposed to
per-partition scale factors consumed by the ACT evictions of attn@V.
"""
import sys
from contextlib import ExitStack

sys.path.insert(0, "/opt/trn_rl_repo")

import numpy as np

P = 128
D = 1024            # d_in = d_k = d_v
S = 2048            # full kv sequence per batch
HS = 1024           # per-core sequence half (own projection rows)
NQ = 1024           # query rows per core
B = 4
KT = D // P         # 8 contraction tiles
ST = S // P         # 16 s tiles
HST = HS // P       # 8 s tiles per half
XC = 512            # projection chunk width
QH = 512            # scores free-dim chunk
QB = 1024           # q block width in attention
JH = 2              # j-tiles per attn half-pass
DVC = 512           # dv chunk width

GROUPS = [[0, 1], [2, 3], [4, 5], [6, 7]]

_CACHE = {}


def _build():
    import concourse.bacc as bacc
    import concourse.mybir as mybir
    import concourse.tile as tile

    F32 = mybir.dt.float32
    F32R = mybir.dt.float32r
    BF16 = mybir.dt.bfloat16
    AF = mybir.ActivationFunctionType

    nc = bacc.Bacc("TRN2", target_bir_lowering=False, debug=False, num_devices=8)

    # f32r is a 4-byte container mapping to np.float32; host pre-rounds the
    # mantissa so landing raw bytes into f32r tiles is exact.
    xtq_d = nc.dram_tensor("xtq", [D, HS], F32R, kind="ExternalInput")
    wq_d = nc.dram_tensor("wq", [D, D], F32R, kind="ExternalInput")
    wk_d = nc.dram_tensor("wk", [D, D], F32R, kind="ExternalInput")
    wv_d = nc.dram_tensor("wv", [D, D], F32R, kind="ExternalInput")
    bqt_d = nc.dram_tensor("bqt", [P, KT], F32, kind="ExternalInput")
    bkt_d = nc.dram_tensor("bkt", [P, KT], F32, kind="ExternalInput")
    bvb_d = nc.dram_tensor("bvb", [P, D], BF16, kind="ExternalInput")
    o_d = nc.dram_tensor("o", [NQ, D], F32, kind="ExternalOutput")

    with tile.TileContext(nc) as tc:
        with (
            tc.tile_pool(name="const", bufs=1) as constp,
            tc.tile_pool(name="qtp", bufs=1) as qtp,
            tc.tile_pool(name="dram", bufs=1, space="DRAM") as dramp,
            tc.tile_pool(name="misc", bufs=1) as miscp,
            tc.tile_pool(name="stg", bufs=2) as stgp,
            tc.tile_pool(name="outp", bufs=2) as outp,
        ):
            bq_sb = constp.tile([P, KT], F32)
            bk_sb = constp.tile([P, KT], F32)
            nc.scalar.dma_start(bq_sb[:], bqt_d.ap())
            nc.scalar.dma_start(bk_sb[:], bkt_d.ap())
            # ones=32 folds the 1/sqrt(d_k)=1/32 output scale into the rowsum
            ones_b = constp.tile([P, 1], BF16)
            nc.vector.memset(ones_b[:], 32.0)
            ident = constp.tile([1, 1], F32)
            nc.vector.memset(ident[:], 1.0)

            QT = qtp.tile([P, KT, NQ], F32R)      # [dk%128, dk//128, q]

            # exchange bounce buffers (group order: even core then odd core)
            kx_in = dramp.tile([D, HS], F32R)
            kx_out = dramp.tile([2, D, HS], F32R)
            vx_in = dramp.tile([HS, D], BF16)
            vx_out = dramp.tile([2, HS, D], BF16)

            xtq_r = xtq_d.ap().rearrange("(t p) q -> p t q", p=P)
            kxi_r = kx_in.rearrange("(t p) s -> p t s", p=P)
            kxo_r = kx_out.rearrange("g (t p) s -> p g t s", p=P)
            vxo_r = vx_out.rearrange("g (sl p) d -> p g sl d", p=P)

            proj_es = ExitStack()
            xp = proj_es.enter_context(tc.tile_pool(name="xp", bufs=1))
            wqp = proj_es.enter_context(tc.tile_pool(name="wqp", bufs=1))
            pp = proj_es.enter_context(tc.tile_pool(name="pp", bufs=8, space="PSUM"))
            wkp_es = ExitStack()
            wkp = wkp_es.enter_context(tc.tile_pool(name="wkp", bufs=1))

            # input loads: x + wk first (K proj), wq behind them (Q proj)
            xr = xp.tile([P, KT, HS], F32R)
            wk_sb = wkp.tile([P, KT, D], F32R)
            wq_sb = wqp.tile([P, KT, D], F32R)
            nc.sync.dma_start(xr[:, :, 0:256], xtq_r[:, :, 0:256])
            nc.scalar.dma_start(xr[:, :, 256:512], xtq_r[:, :, 256:512])
            for t in range(KT):
                eng = nc.sync if t % 2 == 0 else nc.scalar
                eng.dma_start(wk_sb[:, t, :], wk_d.ap()[t * P:(t + 1) * P, :])
            nc.sync.dma_start(xr[:, :, 512:768], xtq_r[:, :, 512:768])
            nc.scalar.dma_start(xr[:, :, 768:1024], xtq_r[:, :, 768:1024])
            for t in range(KT):
                eng = nc.sync if t % 2 == 0 else nc.scalar
                eng.dma_start(wq_sb[:, t, :], wq_d.ap()[t * P:(t + 1) * P, :])

            # ---- K proj (own half): t-outer so matmuls chase the wk tile
            # arrivals; 8 PSUM accumulators live at once ----
            for c in range(HS // XC):
                pss = [pp.tile([P, XC], F32, tag="pp", name="ps")
                       for _ in range(KT)]
                for t in range(KT):
                    for dk in range(KT):
                        nc.tensor.matmul(
                            pss[dk][:], wk_sb[:, t, dk * P:(dk + 1) * P],
                            xr[:, t, c * XC:(c + 1) * XC],
                            start=(t == 0), stop=(t == KT - 1),
                        )
                for dk in range(KT):
                    ks = stgp.tile([P, XC], F32R, tag="kstg", name="ks")
                    nc.scalar.activation(ks[:], pss[dk][:], AF.Identity,
                                         bias=bk_sb[:, dk:dk + 1])
                    nc.gpsimd.dma_start(
                        kxi_r[:, dk, c * XC:(c + 1) * XC], ks[:])

            nc.gpsimd.collective_compute(
                "AllGather", mybir.AluOpType.bypass,
                replica_groups=GROUPS,
                ins=[kx_in.opt()], outs=[kx_out.opt()],
            )
            wkp_es.close()
            # K_sb reuses wk's SBUF footprint (pool lifetimes don't overlap)
            ksb_es = ExitStack()
            ksbp = ksb_es.enter_context(
                tc.tile_pool(name="ksb", bufs=1, side="right"))
            K_sb = ksbp.tile([P, KT, S], F32R)    # [dk%128, dk//128, s] resident
            wvp_es = ExitStack()
            wvp = wvp_es.enter_context(tc.tile_pool(name="wvp", bufs=1))
            wv_sb = wvp.tile([P, KT, D], F32R)
            for t in range(KT):
                eng = nc.sync if t % 2 == 0 else nc.scalar
                eng.dma_start(wv_sb[:, t, :], wv_d.ap()[t * P:(t + 1) * P, :])

            # ---- Q proj ----
            for c in range(NQ // XC):
                for dk in range(KT):
                    ps = pp.tile([P, XC], F32, tag="pp", name="ps")
                    for t in range(KT):
                        nc.tensor.matmul(
                            ps[:], wq_sb[:, t, dk * P:(dk + 1) * P],
                            xr[:, t, c * XC:(c + 1) * XC],
                            start=(t == 0), stop=(t == KT - 1),
                        )
                    nc.scalar.activation(
                        QT[:, dk, c * XC:(c + 1) * XC], ps[:],
                        AF.Identity, bias=bq_sb[:, dk:dk + 1],
                    )

            # K reload (AllGather-gated) rides the sync ring, which is idle
            # from here; issued before V proj in program order but the sync
            # engine just parks on the collective semaphore meanwhile.
            for st in range(ST):
                g, sl = st // HST, st % HST
                nc.sync.dma_start(
                    K_sb[:, :, st * P:(st + 1) * P],
                    kxo_r[:, g, :, sl * P:(sl + 1) * P])

            # ---- V proj (own half): V[s, dv] = x chunk (stationary) @ Wv ----
            for c in range(HS // XC):
                for sh in range(XC // P):
                    for dv in range(D // DVC):
                        ps = pp.tile([P, DVC], F32, tag="pp", name="ps")
                        for t in range(KT):
                            nc.tensor.matmul(
                                ps[:],
                                xr[:, t, c * XC + sh * P:c * XC + (sh + 1) * P],
                                wv_sb[:, t, dv * DVC:(dv + 1) * DVC],
                                start=(t == 0), stop=(t == KT - 1),
                            )
                        vs = stgp.tile([P, DVC], BF16, tag="vstg", name="vs")
                        nc.scalar.copy(vs[:], ps[:])
                        nc.gpsimd.dma_start(
                            vx_in[(c * (XC // P) + sh) * P:
                                  (c * (XC // P) + sh + 1) * P,
                                  dv * DVC:(dv + 1) * DVC],
                            vs[:])

            nc.gpsimd.collective_compute(
                "AllGather", mybir.AluOpType.bypass,
                replica_groups=GROUPS,
                ins=[vx_in.opt()], outs=[vx_out.opt()],
            )
            wvp_es.close()
            proj_es.close()

            # ---- attention ----
            attn_es = ExitStack()
            etp = attn_es.enter_context(tc.tile_pool(name="etp", bufs=1))
            vsb = attn_es.enter_context(tc.tile_pool(name="vsb", bufs=1,
                                                     side="right"))
            eT = etp.tile([P, ST, QB], BF16)      # [s%128, s//128, q]
            V_sb = vsb.tile([P, ST, D], BF16)     # [s%128, s//128, dv]
            bvb_sb = etp.tile([P, D], BF16)
            nc.scalar.dma_start(bvb_sb[:], bvb_d.ap())
            for st in range(ST):
                g, sl = st // HST, st % HST
                nc.sync.dma_start(V_sb[:, st, :], vxo_r[:, g, sl, :])

            pss_es = ExitStack()
            pss = pss_es.enter_context(
                tc.tile_pool(name="pss", bufs=2, space="PSUM"))
            for st in range(ST):
                for qh in range(QB // QH):
                    ps = pss.tile([P, QH], F32, tag="ps", name="ps")
                    for dk in range(KT):
                        nc.tensor.matmul(
                            ps[:],
                            K_sb[:, dk, st * P:(st + 1) * P],
                            QT[:, dk, qh * QH:(qh + 1) * QH],
                            start=(dk == 0), stop=(dk == KT - 1),
                        )
                    nc.scalar.activation(
                        eT[:, st, qh * QH:(qh + 1) * QH], ps[:], AF.Exp)
            pss_es.close()

            with (
                tc.tile_pool(name="pso", bufs=1, space="PSUM") as pso,
                tc.tile_pool(name="psr", bufs=2, space="PSUM") as psr,
                tc.tile_pool(name="pst", bufs=1, space="PSUM") as pst,
            ):
                # rowsum (x32) over s via ones matmul, per q-half
                rec32s = []
                for qh in range(QB // QH):
                    prs = psr.tile([1, QH], F32, tag="prs", name="prs")
                    for st in range(ST):
                        nc.tensor.matmul(
                            prs[:], ones_b[:], eT[:, st, qh * QH:(qh + 1) * QH],
                            start=(st == 0), stop=(st == ST - 1))
                    rec32 = miscp.tile([1, QH], F32, tag=f"rec32{qh}",
                                       name="rec32")
                    nc.vector.reciprocal(rec32[:], prs[:])
                    rec32s.append(rec32)
                # attn @ V in j-half passes: 4 psum accumulators each
                rcs = []
                for jh in range(QB // P // JH):
                    pos = [
                        pso.tile([P, DVC], F32, tag=f"po{u}", name="po")
                        for u in range(JH * (D // DVC))
                    ]
                    for st in range(ST):
                        for ji in range(JH):
                            j = jh * JH + ji
                            for dv in range(D // DVC):
                                nc.tensor.matmul(
                                    pos[ji * (D // DVC) + dv][:],
                                    eT[:, st, j * P:(j + 1) * P],
                                    V_sb[:, st, dv * DVC:(dv + 1) * DVC],
                                    start=(st == 0), stop=(st == ST - 1),
                                )
                    if jh == 0:
                        # emitted after a dense MM batch so the DVE->PE->ACT
                        # reciprocal/transpose chain hides under the matmuls
                        for j in range(QB // P):
                            qh, jq = divmod(j, QH // P)
                            pt = pst.tile([P, 1], F32, tag="pt", name="pt")
                            nc.tensor.transpose(
                                pt[:], rec32s[qh][:, jq * P:(jq + 1) * P],
                                ident[:])
                            rc = miscp.tile([P, 1], F32, tag=f"rc{j}",
                                            name="rc")
                            nc.scalar.copy(rc[:], pt[:])
                            rcs.append(rc)
                    for ji in range(JH):
                        j = jh * JH + ji
                        for dv in range(D // DVC):
                            po = pos[ji * (D // DVC) + dv]
                            osb = outp.tile([P, DVC], F32, tag="osb",
                                            name="osb")
                            nc.scalar.activation(osb[:], po[:], AF.Copy,
                                                 scale=rcs[j][:])
                            nc.vector.tensor_tensor(
                                osb[:], osb[:],
                                bvb_sb[:, dv * DVC:(dv + 1) * DVC],
                                op=mybir.AluOpType.add,
                            )
                            nc.scalar.dma_start(
                                o_d.ap()[j * P:(j + 1) * P,
                                         dv * DVC:(dv + 1) * DVC],
                                osb[:],
                            )
            attn_es.close()
            ksb_es.close()
    nc.compile()
    return nc


def _get_nc():
    if "nc" not in _CACHE:
        _CACHE["nc"] = _build()
    return _CACHE["nc"]


def _preround(a, bits=13):
    # round mantissa to `bits` explicit bits (round-to-nearest) so the
    # device's f32->f32r interpretation is lossless
    u = np.ascontiguousarray(a, dtype=np.float32).view(np.uint32)
    shift = 23 - bits
    add = np.uint32(1 << (shift - 1))
    u = ((u.astype(np.uint64) + add) >> shift << shift).astype(np.uint32)
    return np.ascontiguousarray(u.view(np.float32))


def _in_maps(x, Wq, bq, Wk, bk, Wv, bv):
    import ml_dtypes
    x = _preround(x)
    wq = _preround(Wq)
    wk = _preround(Wk)
    wv = _preround(Wv)
    bqt = np.ascontiguousarray(np.reshape(bq, (KT, P)).T, dtype=np.float32)
    bkt = np.ascontiguousarray(np.reshape(bk, (KT, P)).T, dtype=np.float32)
    bvb = np.ascontiguousarray(
        np.tile(np.asarray(bv, np.float32) / 32.0, (P, 1)).astype(ml_dtypes.bfloat16))
    maps = []
    for c in range(8):
        b, h = c // 2, c % 2
        xtq = np.ascontiguousarray(x[b, h * HS:(h + 1) * HS].T)  # [D, HS]
        maps.append({
            "xtq": xtq, "wq": wq, "wk": wk, "wv": wv,
            "bqt": bqt, "bkt": bkt, "bvb": bvb,
        })
    return maps


def _run(inputs, trace=False, tmpdir=None):
    import time

    from concourse.bass_utils import run_bass_kernel_spmd

    nc = _get_nc()
    maps = _in_maps(**inputs)
    last_err = None
    for attempt in range(3):
        try:
            res = run_bass_kernel_spmd(nc, maps, core_ids=list(range(8)),
                                       trace=trace, tmpdir=tmpdir)
            break
        except Exception as e:  # transient NRT device errors recover on retry
            last_err = e
            time.sleep(10)
    else:
        raise last_err
    out = np.empty((B, S, D), dtype=np.float32)
    for c in range(8):
        b, h = c // 2, c % 2
        out[b, h * NQ:(h + 1) * NQ, :] = res.results[c]["o"]
    return out, res


def kernel(**inputs):
    out, _ = _run(inputs, trace=False)
    return out


# revision 12
# speedup vs baseline: 1.0305x; 1.0305x over previous
"""AttentionBlock kernel for 8 Trainium2 NeuronCores.

Reference computation (per batch b):
    Q = x[b] @ Wq + bq            [S, D]
    K = x[b] @ Wk + bk            [S, D]
    V = x[b] @ Wv + bv            [S, D]
    scores = Q @ K^T              [S, S]   (unscaled)
    attn = softmax(scores, -1)
    out[b] = attn @ V / sqrt(D)

Sharding: 8 cores = 4 batches x 2 sequence-halves. Each core projects
Q/K/V only for its OWN 1024 rows, then the K-half (f32r) and V-half (bf16)
are exchanged within the pair via a 2-way AllGather (DRAM bounce), so the
duplicated K/V projection work of the previous version is gone (-22% PE
work). Attention (scores/softmax/attn@V) runs per-core over the core's
1024 queries x full 2048 keys. AllGather output is in group order
(even-core half first), identical on both cores, so the program stays
SPMD-uniform; key order matches the natural sequence order.

Precision: all score-path operands (x, Wq, Wk, Wv, K) stay f32r. Host
pre-rounds x/W to 13 mantissa bits so the device f32->f32r interpretation
is lossless, which lets DMAs land f32 bytes directly into f32r tiles (no
DVE casts anywhere). eT (=exp scores) and V are bf16 (output-path only,
~1e-3 error), halving their SBUF/DMA cost; K stays fully SBUF-resident
[128, 8, 2048] f32r so there is no K DRAM stream during scores.

Engine plan: sync ring = input loads (x, wk, wq, wv) then the AllGather-
gated K/V reloads; scalar(ACT) = evictions + inline output stores;
gpsimd = exchange bounce stores + the two collectives; DVE nearly idle
(reciprocal + bias adds only). rowsum rides a ones(=32)-matmul on the PE
(folds the 1/sqrt(d_k)=1/32 scale), reciprocal on DVE, PE-transposed to
per-partition scale factors consumed by the ACT evictions of attn@V.
"""
import sys
from contextlib import ExitStack

sys.path.insert(0, "/opt/trn_rl_repo")

import numpy as np

P = 128
D = 1024            # d_in = d_k = d_v
S = 2048            # full kv sequence per batch
HS = 1024           # per-core sequence half (own projection rows)
NQ = 1024           # query rows per core
B = 4
KT = D // P         # 8 contraction tiles
ST = S // P         # 16 s tiles
HST = HS // P       # 8 s tiles per half
XC = 512            # projection chunk width
QH = 512            # scores free-dim chunk
QB = 1024           # q block width in attention
JH = 2              # j-tiles per attn half-pass
DVC = 512           # dv chunk width

GROUPS = [[0, 1], [2, 3], [4, 5], [6, 7]]

_CACHE = {}


def _build():
    import concourse.bacc as bacc
    import concourse.mybir as mybir
    import concourse.tile as tile

    F32 = mybir.dt.float32
    F32R = mybir.dt.float32r
    BF16 = mybir.dt.bfloat16
    AF = mybir.ActivationFunctionType

    nc = bacc.Bacc("TRN2", target_bir_lowering=False, debug=False, num_devices=8)

    # f32r is a 4-byte container mapping to np.float32; host pre-rounds the
    # mantissa so landing raw bytes into f32r tiles is exact.
    xtq_d = nc.dram_tensor("xtq", [D, HS], F32R, kind="ExternalInput")
    wq_d = nc.dram_tensor("wq", [D, D], F32R, kind="ExternalInput")
    wk_d = nc.dram_tensor("wk", [D, D], F32R, kind="ExternalInput")
    wv_d = nc.dram_tensor("wv", [D, D], F32R, kind="ExternalInput")
    bqt_d = nc.dram_tensor("bqt", [P, KT], F32, kind="ExternalInput")
    bkt_d = nc.dram_tensor("bkt", [P, KT], F32, kind="ExternalInput")
    bvb_d = nc.dram_tensor("bvb", [P, D], BF16, kind="ExternalInput")
    o_d = nc.dram_tensor("o", [NQ, D], F32, kind="ExternalOutput")

    with tile.TileContext(nc) as tc:
        with (
            tc.tile_pool(name="const", bufs=1) as constp,
            tc.tile_pool(name="qtp", bufs=1) as qtp,
            tc.tile_pool(name="dram", bufs=1, space="DRAM") as dramp,
            tc.tile_pool(name="misc", bufs=1) as miscp,
            tc.tile_pool(name="stg", bufs=2) as stgp,
            tc.tile_pool(name="outp", bufs=2) as outp,
        ):
            bq_sb = constp.tile([P, KT], F32)
            bk_sb = constp.tile([P, KT], F32)
            nc.scalar.dma_start(bq_sb[:], bqt_d.ap())
            nc.scalar.dma_start(bk_sb[:], bkt_d.ap())
            # ones=32 folds the 1/sqrt(d_k)=1/32 output scale into the rowsum
            ones_b = constp.tile([P, 1], BF16)
            nc.vector.memset(ones_b[:], 32.0)
            ident = constp.tile([1, 1], F32)
            nc.vector.memset(ident[:], 1.0)

            QT = qtp.tile([P, KT, NQ], F32R)      # [dk%128, dk//128, q]

            # exchange bounce buffers (group order: even core then odd core)
            kx_in = dramp.tile([D, HS], F32R)
            kx_out = dramp.tile([2, D, HS], F32R)
            vx_in = dramp.tile([HS, D], BF16)
            vx_out = dramp.tile([2, HS, D], BF16)

            xtq_r = xtq_d.ap().rearrange("(t p) q -> p t q", p=P)
            kxi_r = kx_in.rearrange("(t p) s -> p t s", p=P)
            kxo_r = kx_out.rearrange("g (t p) s -> p g t s", p=P)
            vxo_r = vx_out.rearrange("g (sl p) d -> p g sl d", p=P)

            proj_es = ExitStack()
            xp = proj_es.enter_context(tc.tile_pool(name="xp", bufs=1))
            wqp = proj_es.enter_context(tc.tile_pool(name="wqp", bufs=1))
            pp = proj_es.enter_context(tc.tile_pool(name="pp", bufs=8, space="PSUM"))
            wkp_es = ExitStack()
            wkp = wkp_es.enter_context(tc.tile_pool(name="wkp", bufs=1))

            # input loads: x + wk first (K proj), wq behind them (Q proj)
            xr = xp.tile([P, KT, HS], F32R)
            wk_sb = wkp.tile([P, KT, D], F32R)
            wq_sb = wqp.tile([P, KT, D], F32R)
            nc.sync.dma_start(xr[:, :, 0:256], xtq_r[:, :, 0:256])
            nc.scalar.dma_start(xr[:, :, 256:512], xtq_r[:, :, 256:512])
            for t in range(KT):
                eng = nc.sync if t % 2 == 0 else nc.scalar
                eng.dma_start(wk_sb[:, t, :], wk_d.ap()[t * P:(t + 1) * P, :])
            nc.sync.dma_start(xr[:, :, 512:768], xtq_r[:, :, 512:768])
            nc.scalar.dma_start(xr[:, :, 768:1024], xtq_r[:, :, 768:1024])
            for t in range(KT):
                eng = nc.sync if t % 2 == 0 else nc.scalar
                eng.dma_start(wq_sb[:, t, :], wq_d.ap()[t * P:(t + 1) * P, :])

            # ---- K proj (own half): t-outer so matmuls chase the wk tile
            # arrivals; 8 PSUM accumulators live at once ----
            for c in range(HS // XC):
                pss = [pp.tile([P, XC], F32, tag="pp", name="ps")
                       for _ in range(KT)]
                for t in range(KT):
                    for dk in range(KT):
                        nc.tensor.matmul(
                            pss[dk][:], wk_sb[:, t, dk * P:(dk + 1) * P],
                            xr[:, t, c * XC:(c + 1) * XC],
                            start=(t == 0), stop=(t == KT - 1),
                        )
                for dk in range(KT):
                    ks = stgp.tile([P, XC], F32R, tag="kstg", name="ks")
                    nc.scalar.activation(ks[:], pss[dk][:], AF.Identity,
                                         bias=bk_sb[:, dk:dk + 1])
                    nc.scalar.dma_start(
                        kxi_r[:, dk, c * XC:(c + 1) * XC], ks[:])

            nc.gpsimd.collective_compute(
                "AllGather", mybir.AluOpType.bypass,
                replica_groups=GROUPS,
                ins=[kx_in.opt()], outs=[kx_out.opt()],
            )
            wkp_es.close()
            # K_sb reuses wk's SBUF footprint (pool lifetimes don't overlap)
            ksb_es = ExitStack()
            ksbp = ksb_es.enter_context(
                tc.tile_pool(name="ksb", bufs=1, side="right"))
            K_sb = ksbp.tile([P, KT, S], F32R)    # [dk%128, dk//128, s] resident
            wvp_es = ExitStack()
            wvp = wvp_es.enter_context(tc.tile_pool(name="wvp", bufs=1))
            wv_sb = wvp.tile([P, KT, D], F32R)
            for t in range(KT):
                eng = nc.sync if t % 2 == 0 else nc.scalar
                eng.dma_start(wv_sb[:, t, :], wv_d.ap()[t * P:(t + 1) * P, :])

            # ---- Q proj ----
            for c in range(NQ // XC):
                for dk in range(KT):
                    ps = pp.tile([P, XC], F32, tag="pp", name="ps")
                    for t in range(KT):
                        nc.tensor.matmul(
                            ps[:], wq_sb[:, t, dk * P:(dk + 1) * P],
                            xr[:, t, c * XC:(c + 1) * XC],
                            start=(t == 0), stop=(t == KT - 1),
                        )
                    nc.scalar.activation(
                        QT[:, dk, c * XC:(c + 1) * XC], ps[:],
                        AF.Identity, bias=bq_sb[:, dk:dk + 1],
                    )

            # K reload (AllGather-gated) rides the sync ring, which is idle
            # from here; issued before V proj in program order but the sync
            # engine just parks on the collective semaphore meanwhile.
            for st in range(ST):
                g, sl = st // HST, st % HST
                nc.sync.dma_start(
                    K_sb[:, :, st * P:(st + 1) * P],
                    kxo_r[:, g, :, sl * P:(sl + 1) * P])

            # ---- V proj (own half): V[s, dv] = x chunk (stationary) @ Wv ----
            for c in range(HS // XC):
                for sh in range(XC // P):
                    for dv in range(D // DVC):
                        ps = pp.tile([P, DVC], F32, tag="pp", name="ps")
                        for t in range(KT):
                            nc.tensor.matmul(
                                ps[:],
                                xr[:, t, c * XC + sh * P:c * XC + (sh + 1) * P],
                                wv_sb[:, t, dv * DVC:(dv + 1) * DVC],
                                start=(t == 0), stop=(t == KT - 1),
                            )
                        vs = stgp.tile([P, DVC], BF16, tag="vstg", name="vs")
                        nc.scalar.copy(vs[:], ps[:])
                        nc.scalar.dma_start(
                            vx_in[(c * (XC // P) + sh) * P:
                                  (c * (XC // P) + sh + 1) * P,
                                  dv * DVC:(dv + 1) * DVC],
                            vs[:])

            nc.gpsimd.collective_compute(
                "AllGather", mybir.AluOpType.bypass,
                replica_groups=GROUPS,
                ins=[vx_in.opt()], outs=[vx_out.opt()],
            )
            wvp_es.close()
            proj_es.close()

            # ---- attention ----
            attn_es = ExitStack()
            etp = attn_es.enter_context(tc.tile_pool(name="etp", bufs=1))
            vsb = attn_es.enter_context(tc.tile_pool(name="vsb", bufs=1,
                                                     side="right"))
            eT = etp.tile([P, ST, QB], BF16)      # [s%128, s//128, q]
            V_sb = vsb.tile([P, ST, D], BF16)     # [s%128, s//128, dv]
            bvb_sb = etp.tile([P, D], BF16)
            nc.scalar.dma_start(bvb_sb[:], bvb_d.ap())
            for st in range(ST):
                g, sl = st // HST, st % HST
                nc.sync.dma_start(V_sb[:, st, :], vxo_r[:, g, sl, :])

            pss_es = ExitStack()
            pss = pss_es.enter_context(
                tc.tile_pool(name="pss", bufs=2, space="PSUM"))
            for st in range(ST):
                for qh in range(QB // QH):
                    ps = pss.tile([P, QH], F32, tag="ps", name="ps")
                    for dk in range(KT):
                        nc.tensor.matmul(
                            ps[:],
                            K_sb[:, dk, st * P:(st + 1) * P],
                            QT[:, dk, qh * QH:(qh + 1) * QH],
                            start=(dk == 0), stop=(dk == KT - 1),
                        )
                    nc.scalar.activation(
                        eT[:, st, qh * QH:(qh + 1) * QH], ps[:], AF.Exp)
            pss_es.close()

            with (
                tc.tile_pool(name="pso", bufs=1, space="PSUM") as pso,
                tc.tile_pool(name="psr", bufs=2, space="PSUM") as psr,
                tc.tile_pool(name="pst", bufs=1, space="PSUM") as pst,
            ):
                # rowsum (x32) over s via ones matmul, per q-half
                rec32s = []
                for qh in range(QB // QH):
                    prs = psr.tile([1, QH], F32, tag="prs", name="prs")
                    for st in range(ST):
                        nc.tensor.matmul(
                            prs[:], ones_b[:], eT[:, st, qh * QH:(qh + 1) * QH],
                            start=(st == 0), stop=(st == ST - 1))
                    rec32 = miscp.tile([1, QH], F32, tag=f"rec32{qh}",
                                       name="rec32")
                    nc.vector.reciprocal(rec32[:], prs[:])
                    rec32s.append(rec32)
                # attn @ V in j-half passes: 4 psum accumulators each
                rcs = []
                for jh in range(QB // P // JH):
                    pos = [
                        pso.tile([P, DVC], F32, tag=f"po{u}", name="po")
                        for u in range(JH * (D // DVC))
                    ]
                    for st in range(ST):
                        for ji in range(JH):
                            j = jh * JH + ji
                            for dv in range(D // DVC):
                                nc.tensor.matmul(
                                    pos[ji * (D // DVC) + dv][:],
                                    eT[:, st, j * P:(j + 1) * P],
                                    V_sb[:, st, dv * DVC:(dv + 1) * DVC],
                                    start=(st == 0), stop=(st == ST - 1),
                                )
                    if jh == 0:
                        # emitted after a dense MM batch so the DVE->PE->ACT
                        # reciprocal/transpose chain hides under the matmuls
                        for j in range(QB // P):
                            qh, jq = divmod(j, QH // P)
                            pt = pst.tile([P, 1], F32, tag="pt", name="pt")
                            nc.tensor.transpose(
                                pt[:], rec32s[qh][:, jq * P:(jq + 1) * P],
                                ident[:])
                            rc = miscp.tile([P, 1], F32, tag=f"rc{j}",
                                            name="rc")
                            nc.scalar.copy(rc[:], pt[:])
                            rcs.append(rc)
                    for ji in range(JH):
                        j = jh * JH + ji
                        for dv in range(D // DVC):
                            po = pos[ji * (D // DVC) + dv]
                            osb = outp.tile([P, DVC], F32, tag="osb",
                                            name="osb")
                            nc.scalar.activation(osb[:], po[:], AF.Copy,
                                                 scale=rcs[j][:])
                            nc.vector.tensor_tensor(
                                osb[:], osb[:],
                                bvb_sb[:, dv * DVC:(dv + 1) * DVC],
                                op=mybir.AluOpType.add,
                            )
                            nc.scalar.dma_start(
                                o_d.ap()[j * P:(j + 1) * P,
                                         dv * DVC:(dv + 1) * DVC],
                                osb[:],
                            )
            attn_es.close()
            ksb_es.close()
    nc.compile()
    return nc


def _get_nc():
    if "nc" not in _CACHE:
        _CACHE["nc"] = _build()
    return _CACHE["nc"]


def _preround(a, bits=13):
    # round mantissa to `bits` explicit bits (round-to-nearest) so the
    # device's f32->f32r interpretation is lossless
    u = np.ascontiguousarray(a, dtype=np.float32).view(np.uint32)
    shift = 23 - bits
    add = np.uint32(1 << (shift - 1))
    u = ((u.astype(np.uint64) + add) >> shift << shift).astype(np.uint32)
    return np.ascontiguousarray(u.view(np.float32))


def _in_maps(x, Wq, bq, Wk, bk, Wv, bv):
    import ml_dtypes
    x = _preround(x)
    wq = _preround(Wq)
    wk = _preround(Wk)
    wv = _preround(Wv)
    bqt = np.ascontiguousarray(np.reshape(bq, (KT, P)).T, dtype=np.float32)
    bkt = np.ascontiguousarray(np.reshape(bk, (KT, P)).T, dtype=np.float32)
    bvb = np.ascontiguousarray(
        np.tile(np.asarray(bv, np.float32) / 32.0, (P, 1)).astype(ml_dtypes.bfloat16))
    maps = []
    for c in range(8):
        b, h = c // 2, c % 2
        xtq = np.ascontiguousarray(x[b, h * HS:(h + 1) * HS].T)  # [D, HS]
        maps.append({
            "xtq": xtq, "wq": wq, "wk": wk, "wv": wv,
            "bqt": bqt, "bkt": bkt, "bvb": bvb,
        })
    return maps


def _run(inputs, trace=False, tmpdir=None):
    import time

    from concourse.bass_utils import run_bass_kernel_spmd

    nc = _get_nc()
    maps = _in_maps(**inputs)
    last_err = None
    for attempt in range(3):
        try:
            res = run_bass_kernel_spmd(nc, maps, core_ids=list(range(8)),
                                       trace=trace, tmpdir=tmpdir)
            break
        except Exception as e:  # transient NRT device errors recover on retry
            last_err = e
            time.sleep(10)
    else:
        raise last_err
    out = np.empty((B, S, D), dtype=np.float32)
    for c in range(8):
        b, h = c // 2, c % 2
        out[b, h * NQ:(h + 1) * NQ, :] = res.results[c]["o"]
    return out, res


def kernel(**inputs):
    out, _ = _run(inputs, trace=False)
    return out


# revision 13
# speedup vs baseline: 1.2877x; 1.2497x over previous
"""AttentionBlock kernel for 8 Trainium2 NeuronCores.

Reference computation (per batch b):
    Q = x[b] @ Wq + bq            [S, D]
    K = x[b] @ Wk + bk            [S, D]
    V = x[b] @ Wv + bv            [S, D]
    scores = Q @ K^T              [S, S]   (unscaled)
    attn = softmax(scores, -1)
    out[b] = attn @ V / sqrt(D)

Sharding: 8 cores = 4 batches x 2 query-halves. Each core computes K for
its batch's FULL sequence (duplicated within the pair - the pairwise
AllGather measured ~70GB/s effective, far too slow to exchange K's 12MB
bounce+reload), but V is projected only for the core's own half and
exchanged via a 2MB-in AllGather that hides under the scores phase.
Attention runs per-core over 1024 queries x 2048 keys.

Precision: score-path operands (x, Wq, Wk, K, QT) stay f32r. Host
pre-rounds x/W to 13 mantissa bits so the device's f32->f32r reading is
lossless, letting DMAs land raw f32 bytes directly into f32r tiles - no
DVE casts anywhere. eT (=exp scores) and V are bf16 (output-path only,
~1e-3 error). K^T [128,8,2048] f32r is written straight from the ACT
evictions of the K projection and stays SBUF-resident - the baseline's
16MB K DRAM roundtrip and its re-round casts are gone entirely.

SBUF is the binding constraint (192KB/partition usable): K 64K + QT 32K
persistent, one 32K weight pool rotating wk->wv->wq, one 32K x-stream
pool (4 xt chunks for K, then xtq twice for V and Q). V/eT bf16 halve
the attention-phase tiles. PE work: 1056 matmuls x 512 cols ~= 258us.

Engine plan: sync+scalar rings carry the streamed loads, V bounce-out
(sync) and output stores (scalar, inline after eviction); gpsimd only
triggers the V AllGather. rowsum rides a ones(=32)-matmul on the PE
(folding the 1/sqrt(d_k)=1/32 scale), reciprocal on DVE, PE-transposed
to per-partition scales consumed by the ACT evictions of attn@V.
"""
import sys
from contextlib import ExitStack

sys.path.insert(0, "/opt/trn_rl_repo")

import numpy as np

P = 128
D = 1024            # d_in = d_k = d_v
S = 2048            # full kv sequence per batch
HS = 1024           # per-core half (own V rows / own queries)
NQ = 1024           # query rows per core
B = 4
KT = D // P         # 8 contraction tiles
ST = S // P         # 16 s tiles
HST = HS // P       # 8 s tiles per half
XC = 512            # x streaming chunk width
QH = 512            # scores free-dim chunk
QB = 1024           # q block width in attention
JH = 2              # j-tiles per attn half-pass
DVC = 512           # dv chunk width

GROUPS = [[0, 1], [2, 3], [4, 5], [6, 7]]

_CACHE = {}


def _build():
    import concourse.bacc as bacc
    import concourse.mybir as mybir
    import concourse.tile as tile

    F32 = mybir.dt.float32
    F32R = mybir.dt.float32r
    BF16 = mybir.dt.bfloat16
    AF = mybir.ActivationFunctionType

    nc = bacc.Bacc("TRN2", target_bir_lowering=False, debug=False, num_devices=8)

    # f32r is a 4-byte container mapping to np.float32; host pre-rounds the
    # mantissa so landing raw bytes into f32r tiles is exact.
    xt_d = nc.dram_tensor("xt", [D, S], F32R, kind="ExternalInput")
    xtq_d = nc.dram_tensor("xtq", [D, HS], F32R, kind="ExternalInput")
    wq_d = nc.dram_tensor("wq", [D, D], F32R, kind="ExternalInput")
    wk_d = nc.dram_tensor("wk", [D, D], F32R, kind="ExternalInput")
    wv_d = nc.dram_tensor("wv", [D, D], F32R, kind="ExternalInput")
    bqt_d = nc.dram_tensor("bqt", [P, KT], F32, kind="ExternalInput")
    bkt_d = nc.dram_tensor("bkt", [P, KT], F32, kind="ExternalInput")
    bvb_d = nc.dram_tensor("bvb", [P, D], BF16, kind="ExternalInput")
    o_d = nc.dram_tensor("o", [NQ, D], F32, kind="ExternalOutput")

    with tile.TileContext(nc) as tc:
        with (
            tc.tile_pool(name="const", bufs=1) as constp,
            tc.tile_pool(name="qtp", bufs=1) as qtp,
            tc.tile_pool(name="ksb", bufs=1, side="right") as ksbp,
            tc.tile_pool(name="dram", bufs=1, space="DRAM") as dramp,
            tc.tile_pool(name="misc", bufs=1) as miscp,
            tc.tile_pool(name="stg", bufs=3) as stgp,
            tc.tile_pool(name="outp", bufs=2) as outp,
        ):
            bq_sb = constp.tile([P, KT], F32)
            bk_sb = constp.tile([P, KT], F32)
            nc.scalar.dma_start(bq_sb[:], bqt_d.ap())
            nc.scalar.dma_start(bk_sb[:], bkt_d.ap())
            # ones=32 folds the 1/sqrt(d_k)=1/32 output scale into the rowsum
            ones_b = constp.tile([P, 1], BF16)
            nc.vector.memset(ones_b[:], 32.0)
            ident = constp.tile([1, 1], F32)
            nc.vector.memset(ident[:], 1.0)

            QT = qtp.tile([P, KT, NQ], F32R)      # [dk%128, dk//128, q]
            K_sb = ksbp.tile([P, KT, S], F32R)    # [dk%128, dk//128, s] resident

            # V exchange bounce (group order: even core half, odd core half)
            vx_in = dramp.tile([HS, D], BF16)
            vx_out = dramp.tile([2, HS, D], BF16)

            xt_r = xt_d.ap().rearrange("(t p) s -> p t s", p=P)
            xtq_r = xtq_d.ap().rearrange("(t p) q -> p t q", p=P)
            vxo_r = vx_out.rearrange("g (sl p) d -> p g sl d", p=P)

            proj_es = ExitStack()
            wp = proj_es.enter_context(tc.tile_pool(name="wp", bufs=1))
            xlp = proj_es.enter_context(tc.tile_pool(name="xlp", bufs=2))
            pp = proj_es.enter_context(tc.tile_pool(name="pp", bufs=8, space="PSUM"))

            def load_w(w_d):
                # one rotating 32KB slot: wk -> wv -> wq
                w_sb = wp.tile([P, KT, D], F32R, tag="w", name="w")
                for t in range(KT):
                    eng = nc.sync if t % 2 == 0 else nc.scalar
                    eng.dma_start(w_sb[:, t, :], w_d.ap()[t * P:(t + 1) * P, :])
                return w_sb

            def load_x(x_r, c):
                xc = xlp.tile([P, KT, XC], F32R, tag="x", name="xc")
                half = XC // 2
                lo = c * XC
                nc.sync.dma_start(xc[:, :, 0:half], x_r[:, :, lo:lo + half])
                nc.scalar.dma_start(xc[:, :, half:], x_r[:, :, lo + half:lo + XC])
                return xc

            # ---- K proj (full sequence): t-outer so matmuls chase the wk
            # tile arrivals; evictions write K_sb directly ----
            xc0 = load_x(xt_r, 0)
            wk_sb = load_w(wk_d)
            xc1 = load_x(xt_r, 1)
            xcs = [xc0, xc1]
            for c in range(S // XC):
                xc = xcs[c] if c < 2 else load_x(xt_r, c)
                pss = [pp.tile([P, XC], F32, tag="pp", name="ps")
                       for _ in range(KT)]
                for t in range(KT):
                    for dk in range(KT):
                        nc.tensor.matmul(
                            pss[dk][:], wk_sb[:, t, dk * P:(dk + 1) * P],
                            xc[:, t, :],
                            start=(t == 0), stop=(t == KT - 1),
                        )
                for dk in range(KT):
                    nc.scalar.activation(
                        K_sb[:, dk, c * XC:(c + 1) * XC], pss[dk][:],
                        AF.Identity, bias=bk_sb[:, dk:dk + 1])

            # ---- V proj (own half): V[s, dv] = x chunk (stationary) @ Wv ----
            wv_sb = load_w(wv_d)
            for c in range(HS // XC):
                xc = load_x(xtq_r, c)
                for sh in range(XC // P):
                    for dv in range(D // DVC):
                        ps = pp.tile([P, DVC], F32, tag="pp", name="ps")
                        for t in range(KT):
                            nc.tensor.matmul(
                                ps[:],
                                xc[:, t, sh * P:(sh + 1) * P],
                                wv_sb[:, t, dv * DVC:(dv + 1) * DVC],
                                start=(t == 0), stop=(t == KT - 1),
                            )
                        vs = stgp.tile([P, DVC], BF16, tag="vstg", name="vs")
                        nc.scalar.copy(vs[:], ps[:])
                        nc.sync.dma_start(
                            vx_in[(c * (XC // P) + sh) * P:
                                  (c * (XC // P) + sh + 1) * P,
                                  dv * DVC:(dv + 1) * DVC],
                            vs[:])

            nc.gpsimd.collective_compute(
                "AllGather", mybir.AluOpType.bypass,
                replica_groups=GROUPS,
                ins=[vx_in.opt()], outs=[vx_out.opt()],
            )

            # ---- Q proj ----
            wq_sb = load_w(wq_d)
            for c in range(NQ // XC):
                xc = load_x(xtq_r, c)
                for dk in range(KT):
                    ps = pp.tile([P, XC], F32, tag="pp", name="ps")
                    for t in range(KT):
                        nc.tensor.matmul(
                            ps[:], wq_sb[:, t, dk * P:(dk + 1) * P],
                            xc[:, t, :],
                            start=(t == 0), stop=(t == KT - 1),
                        )
                    nc.scalar.activation(
                        QT[:, dk, c * XC:(c + 1) * XC], ps[:],
                        AF.Identity, bias=bq_sb[:, dk:dk + 1],
                    )
            proj_es.close()

            # ---- attention ----
            attn_es = ExitStack()
            etp = attn_es.enter_context(tc.tile_pool(name="etp", bufs=1))
            vsb = attn_es.enter_context(tc.tile_pool(name="vsb", bufs=1,
                                                     side="right"))
            eT = etp.tile([P, ST, QB], BF16)      # [s%128, s//128, q]
            V_sb = vsb.tile([P, ST, D], BF16)     # [s%128, s//128, dv]
            bvb_sb = etp.tile([P, D], BF16)
            nc.scalar.dma_start(bvb_sb[:], bvb_d.ap())
            # V reload rides sync, which parks on the AllGather semaphore;
            # st-sliced so attn@V can consume slabs as they land
            for st in range(ST):
                g, sl = st // HST, st % HST
                nc.sync.dma_start(V_sb[:, st, :], vxo_r[:, g, sl, :])

            pss_es = ExitStack()
            pss = pss_es.enter_context(
                tc.tile_pool(name="pss", bufs=2, space="PSUM"))
            for st in range(ST):
                for qh in range(QB // QH):
                    ps = pss.tile([P, QH], F32, tag="ps", name="ps")
                    for dk in range(KT):
                        nc.tensor.matmul(
                            ps[:],
                            K_sb[:, dk, st * P:(st + 1) * P],
                            QT[:, dk, qh * QH:(qh + 1) * QH],
                            start=(dk == 0), stop=(dk == KT - 1),
                        )
                    nc.scalar.activation(
                        eT[:, st, qh * QH:(qh + 1) * QH], ps[:], AF.Exp)
            pss_es.close()

            with (
                tc.tile_pool(name="pso", bufs=1, space="PSUM") as pso,
                tc.tile_pool(name="psr", bufs=2, space="PSUM") as psr,
                tc.tile_pool(name="pst", bufs=1, space="PSUM") as pst,
            ):
                # rowsum (x32) over s via ones matmul, per q-half
                rec32s = []
                for qh in range(QB // QH):
                    prs = psr.tile([1, QH], F32, tag="prs", name="prs")
                    for st in range(ST):
                        nc.tensor.matmul(
                            prs[:], ones_b[:], eT[:, st, qh * QH:(qh + 1) * QH],
                            start=(st == 0), stop=(st == ST - 1))
                    rec32 = miscp.tile([1, QH], F32, tag=f"rec32{qh}",
                                       name="rec32")
                    nc.vector.reciprocal(rec32[:], prs[:])
                    rec32s.append(rec32)
                # attn @ V in j-half passes: 4 psum accumulators each
                rcs = []
                for jh in range(QB // P // JH):
                    pos = [
                        pso.tile([P, DVC], F32, tag=f"po{u}", name="po")
                        for u in range(JH * (D // DVC))
                    ]
                    for st in range(ST):
                        for ji in range(JH):
                            j = jh * JH + ji
                            for dv in range(D // DVC):
                                nc.tensor.matmul(
                                    pos[ji * (D // DVC) + dv][:],
                                    eT[:, st, j * P:(j + 1) * P],
                                    V_sb[:, st, dv * DVC:(dv + 1) * DVC],
                                    start=(st == 0), stop=(st == ST - 1),
                                )
                    if jh == 0:
                        # emitted after a dense MM batch so the DVE->PE->ACT
                        # reciprocal/transpose chain hides under the matmuls
                        for j in range(QB // P):
                            qh, jq = divmod(j, QH // P)
                            pt = pst.tile([P, 1], F32, tag="pt", name="pt")
                            nc.tensor.transpose(
                                pt[:], rec32s[qh][:, jq * P:(jq + 1) * P],
                                ident[:])
                            rc = miscp.tile([P, 1], F32, tag=f"rc{j}",
                                            name="rc")
                            nc.scalar.copy(rc[:], pt[:])
                            rcs.append(rc)
                    for ji in range(JH):
                        j = jh * JH + ji
                        for dv in range(D // DVC):
                            po = pos[ji * (D // DVC) + dv]
                            osb = outp.tile([P, DVC], F32, tag="osb",
                                            name="osb")
                            nc.scalar.activation(osb[:], po[:], AF.Copy,
                                                 scale=rcs[j][:])
                            nc.vector.tensor_tensor(
                                osb[:], osb[:],
                                bvb_sb[:, dv * DVC:(dv + 1) * DVC],
                                op=mybir.AluOpType.add,
                            )
                            nc.scalar.dma_start(
                                o_d.ap()[j * P:(j + 1) * P,
                                         dv * DVC:(dv + 1) * DVC],
                                osb[:],
                            )
            attn_es.close()
    nc.compile()
    return nc


def _get_nc():
    if "nc" not in _CACHE:
        _CACHE["nc"] = _build()
    return _CACHE["nc"]


def _preround(a, bits=13):
    # round mantissa to `bits` explicit bits (round-to-nearest) so the
    # device's f32->f32r interpretation is lossless
    u = np.ascontiguousarray(a, dtype=np.float32).view(np.uint32)
    shift = 23 - bits
    add = np.uint32(1 << (shift - 1))
    u = ((u.astype(np.uint64) + add) >> shift << shift).astype(np.uint32)
    return np.ascontiguousarray(u.view(np.float32))


def _in_maps(x, Wq, bq, Wk, bk, Wv, bv):
    import ml_dtypes
    x = _preround(x)
    wq = _preround(Wq)
    wk = _preround(Wk)
    wv = _preround(Wv)
    bqt = np.ascontiguousarray(np.reshape(bq, (KT, P)).T, dtype=np.float32)
    bkt = np.ascontiguousarray(np.reshape(bk, (KT, P)).T, dtype=np.float32)
    bvb = np.ascontiguousarray(
        np.tile(np.asarray(bv, np.float32) / 32.0, (P, 1)).astype(ml_dtypes.bfloat16))
    maps = []
    for c in range(8):
        b, h = c // 2, c % 2
        xt = np.ascontiguousarray(x[b].T)                        # [D, S]
        xtq = np.ascontiguousarray(x[b, h * HS:(h + 1) * HS].T)  # [D, HS]
        maps.append({
            "xt": xt, "xtq": xtq, "wq": wq, "wk": wk, "wv": wv,
            "bqt": bqt, "bkt": bkt, "bvb": bvb,
        })
    return maps


def _run(inputs, trace=False, tmpdir=None):
    import time

    from concourse.bass_utils import run_bass_kernel_spmd

    nc = _get_nc()
    maps = _in_maps(**inputs)
    last_err = None
    for attempt in range(3):
        try:
            res = run_bass_kernel_spmd(nc, maps, core_ids=list(range(8)),
                                       trace=trace, tmpdir=tmpdir)
            break
        except Exception as e:  # transient NRT device errors recover on retry
            last_err = e
            time.sleep(10)
    else:
        raise last_err
    out = np.empty((B, S, D), dtype=np.float32)
    for c in range(8):
        b, h = c // 2, c % 2
        out[b, h * NQ:(h + 1) * NQ, :] = res.results[c]["o"]
    return out, res


def kernel(**inputs):
    out, _ = _run(inputs, trace=False)
    return out
